# revision 9
# baseline (speedup 1.0000x reference)
"""Trainium2 Bass kernel for nn_CubicSplineLayer (histogram_binning).

The whole layer collapses to a scalar piecewise-cubic function of x:

    out(x) = (basis(x) - mean) @ W.T + b  =  f(x)

where f is the natural cubic spline through (knots, W) plus the constant
(b - mean.W).  In truncated-power form (exact for the C^2 natural spline
with linear extrapolation, as the reference implements):

    f(x) = K0 + sb*min(x, t9) + sa*relu(x - t9)
           + sum_{j=0}^{8} d_j * relu(min(x, t9) - t_j)^3

(The kink at t9 vanishes because min(x,t9) freezes the spline there; the
reference's odd F[9,1] "above" branch term is exactly zero since F's last
row is zeros.)

Device strategy: pure data-parallel over 8 cores.  Per core the chain is
evaluated with 10 custom DVE ops (1 seed + 9 cubic-kink MACs), each a
single 7-stage fused vector instruction, overlapped with HBM DMA.
"""

import numpy as np

N_CORES = 8
P = 128           # SBUF partitions
FD = 3920         # free elements per partition per core
FD_TILE = 980     # tile free-dim (4 tiles per core)
NPAD = N_CORES * P * FD  # 4,014,080 >= 4,000,000

_SEED_NAME = "ANT_SPLINE_SEED"
_KNOT_NAME = "ANT_SPLINE_KNOT"


def _register_ops():
    """Register the two custom DVE ops in concourse's registry (idempotent).

    SEED:  out = min(x, t9)*sb + K0 + relu(x - t9)*sa     (sa via C3 spill)
    KNOT:  out = acc + relu(min(x, t9) - tj)^3 * dj
    """
    import concourse.dve_ops as dvo

    if _SEED_NAME in dvo._SUB_OPCODE_FOR_NAME:
        return dvo
    from concourse.dve_spec import (
        C0, C1, C2, C3, Spec, Src0, Src1, Zero,
        _has_src1, _spill_c3_to_src1, lower, maxx, minn,
    )
    from concourse.dve_uop import DveOpSpec

    def _seed_ref(in0, in1, s0, s1, imm2):
        x = in0.astype(np.float32)
        return (np.minimum(x, imm2) * s0 + s1) + np.maximum(x - imm2, 0.0) * in1

    # min(Src0,C2)*C0 + C1 + max(Src0-C2,0)*C3   -- 7 ALU stages
    seed_body = _spill_c3_to_src1(
        (minn(Src0, C2) * C0 + C1) + maxx(Src0 - C2, Zero) * C3
    )
    seed_spec = Spec(body=seed_body, reference=_seed_ref)

    def _knot_ref(in0, in1, s0, s1, imm2):
        u = np.maximum(np.minimum(in1.astype(np.float32), imm2) - s0, 0.0)
        return in0.astype(np.float32) + (u * u) * u * s1

    # Src0 + cube(relu(min(Src1,C2) - C0)) * C1  -- 7 ALU stages
    u = maxx(minn(Src1, C2) - C0, Zero)
    knot_spec = Spec(body=Src0 + (u * u) * u * C1, reference=_knot_ref)

    for name, spec in ((_SEED_NAME, seed_spec), (_KNOT_NAME, knot_spec)):
        row = dvo._CUSTOM_DVE_ROW_BASE + len(dvo.OPS)
        assert row < 0x20
        shas = {}
        for ver in ("v3", "v4"):
            s = DveOpSpec(
                name=name, opcode=row, uops=lower(spec, ver=ver),
                rd1_en=_has_src1(spec),
            )
            shas[ver] = s.sha(ver)
        op = dvo.DveOp(name, spec, subdim=False, uops_sha=shas)
        dvo.OPS.append(op)
        dvo._SUB_OPCODE_FOR_NAME[name] = row
        dvo.CUSTOM_DVE_SPECS[name] = spec
    return dvo


def _spline_consts(knots, F, W, b, mean):
    """Host-side (float64) derivation of the truncated-power coefficients."""
    knots = np.asarray(knots, np.float64)
    F = np.asarray(F, np.float64)
    w = np.asarray(W, np.float64)[0]
    b = np.asarray(b, np.float64)
    mean = np.asarray(mean, np.float64)[0]

    h = np.diff(knots)
    gamma = F @ w                       # natural-spline second derivatives
    sb = (w[1] - w[0]) / h[0] - h[0] * gamma[1] / 6.0
    sa = (w[-1] - w[-2]) / h[-1] + h[-1] * gamma[-2] / 6.0
    fppp = (gamma[1:] - gamma[:-1]) / h  # f''' on each piece
    d = np.empty(9)
    d[0] = fppp[0] / 6.0
    d[1:] = (fppp[1:] - fppp[:-1]) / 6.0
    K0 = (b[0] - mean @ w) + w[0] - sb * knots[0]
    t9 = knots[-1]
    return (
        float(sb), float(sa), float(K0), float(t9),
        [float(t) for t in knots[:9]], [float(v) for v in d],
    )


def _build_nc(consts, fd=FD, fd_tile=FD_TILE):
    """Raw Bass, standard BIR ops only (this walrus build rejects every
    raw-ISA instruction, incl. custom DVE ops and Tile's RANGE_CLEAR).

    Per tile t:  DVE: y=min(x,t9); acc=y*sb+K0; r=relu(x-t9);
    acc+=sa*r; per knot j: m=q_j*u_j (=u^3); acc+=d_j*m  -- where the
    scalar engine supplies u_j=Relu(y-t_j), q_j=Square(u_j).
    Double-buffered across 2 parities with per-slot DMA semaphores and
    per-engine op-counter semaphores (s_dv, s_ac) for all RAW/WAR deps."""
    from contextlib import ExitStack

    import concourse.bass as bass
    import concourse.mybir as mybir

    sb, sa, K0, t9, tj, dj = consts
    f32 = mybir.dt.float32
    alu = mybir.AluOpType
    act = mybir.ActivationFunctionType
    T = fd // fd_tile
    assert T * fd_tile == fd
    NK = 9
    DOP = 4 + 2 * NK   # DVE ops per tile
    AOP = 2 * NK       # ACT ops per tile

    nc = bass.Bass(trn_type="TRN2")
    x_in = nc.dram_tensor("x", [P, fd], f32, kind="ExternalInput")
    out = nc.dram_tensor("out", [P, fd], f32, kind="ExternalOutput")

    # ACT bias operands must be pre-registered const APs
    for _i, _v in enumerate(dict.fromkeys(float(-t) for t in tj)):
        if (f32, _v) not in nc.const_aps.aps:
            _t = nc.alloc_sbuf_tensor(f"constk-{_i}", [P, 1], f32)
            nc.gpsimd.memset(_t.ap(), _v)
            nc.const_aps.aps[(f32, _v)] = _t.ap()
    nc.all_engine_barrier()

    with ExitStack() as ctx:
        e = ctx.enter_context
        xb = [e(nc.sbuf_tensor(f"xb{i}", [P, fd_tile], f32)) for i in range(2)]
        yb = [e(nc.sbuf_tensor(f"yb{i}", [P, fd_tile], f32)) for i in range(2)]
        rb = [e(nc.sbuf_tensor(f"rb{i}", [P, fd_tile], f32)) for i in range(2)]
        mb = [e(nc.sbuf_tensor(f"mb{i}", [P, fd_tile], f32)) for i in range(2)]
        acc = [[e(nc.sbuf_tensor(f"acc{i}_{w}", [P, fd_tile], f32))
                for w in range(2)] for i in range(2)]
        ub = [[e(nc.sbuf_tensor(f"ub{i}_{j}", [P, fd_tile], f32))
               for j in range(NK)] for i in range(2)]
        qb = [[e(nc.sbuf_tensor(f"qb{i}_{j}", [P, fd_tile], f32))
               for j in range(NK)] for i in range(2)]
        s_ld = [e(nc.semaphore(f"s_ld{i}")) for i in range(2)]
        s_st = [e(nc.semaphore(f"s_st{i}")) for i in range(2)]
        s_dv = e(nc.semaphore("s_dv"))
        s_ac = e(nc.semaphore("s_ac"))
        blk = e(nc.Block())

        @blk.sync
        def _(sync):
            for t in range(T):
                p = t % 2
                if t >= 2:
                    sync.wait_ge(s_dv, DOP * (t - 1))  # xb[p] free
                sync.dma_start(xb[p][:], x_in[:, t * fd_tile:(t + 1) * fd_tile]
                               ).then_inc(s_ld[p], 16)
                if t >= 1:
                    q = (t - 1) % 2
                    sync.wait_ge(s_dv, DOP * t)
                    sync.dma_start(out[:, (t - 1) * fd_tile:t * fd_tile],
                                   acc[q][0][:]).then_inc(s_st[q], 16)
            q = (T - 1) % 2
            sync.wait_ge(s_dv, DOP * T)
            sync.dma_start(out[:, (T - 1) * fd_tile:T * fd_tile],
                           acc[q][0][:]).then_inc(s_st[q], 16)
            sync.wait_ge(s_st[0], 16 * ((T + 1) // 2))
            sync.wait_ge(s_st[1], 16 * (T // 2))

        @blk.vector
        def _(vector):
            g = 0

            def dv(ins):
                nonlocal g
                ins.then_inc(s_dv, 1)
                g += 1

            for t in range(T):
                p = t % 2
                k = t // 2
                vector.wait_ge(s_ld[p], 16 * (k + 1))
                if t >= 1:
                    vector.wait_ge(s_ac, AOP * t)      # yb/rb[p] readers done
                if t >= 2:
                    vector.wait_ge(s_st[p], 16 * k)    # acc slots free
                if g:
                    vector.wait_ge(s_dv, g)
                dv(nc.vector.tensor_scalar_min(yb[p][:], xb[p][:], t9))
                vector.wait_ge(s_dv, g)
                dv(nc.vector.tensor_scalar(acc[p][0][:], yb[p][:], sb, K0,
                                           alu.mult, alu.add))
                vector.wait_ge(s_dv, g)
                dv(nc.vector.tensor_scalar(rb[p][:], xb[p][:], t9, t9,
                                           alu.max, alu.subtract))
                vector.wait_ge(s_dv, g)
                dv(nc.vector.scalar_tensor_tensor(
                    acc[p][1][:], rb[p][:], sa, acc[p][0][:],
                    alu.mult, alu.add))
                w = 0  # acc[p][1] holds latest
                for j in range(NK):
                    vector.wait_ge(s_dv, g)
                    vector.wait_ge(s_ac, AOP * t + 2 * (j + 1))
                    dv(nc.vector.tensor_tensor(
                        mb[p][:], qb[p][j][:], ub[p][j][:], alu.mult))
                    vector.wait_ge(s_dv, g)
                    dv(nc.vector.scalar_tensor_tensor(
                        acc[p][w][:], mb[p][:], dj[j], acc[p][1 - w][:],
                        alu.mult, alu.add))
                    w = 1 - w
                # after 9 knots (odd count), latest is acc[p][0]

        @blk.scalar
        def _(scalar):
            a = 0
            for t in range(T):
                p = t % 2
                scalar.wait_ge(s_dv, DOP * t + 1)      # y_t written
                for j in range(NK):
                    if a:
                        scalar.wait_ge(s_ac, a)
                    nc.scalar.activation(ub[p][j][:], yb[p][:], act.Relu,
                                         bias=-tj[j]).then_inc(s_ac, 1)
                    a += 1
                    scalar.wait_ge(s_ac, a)
                    nc.scalar.activation(qb[p][j][:], ub[p][j][:], act.Square
                                         ).then_inc(s_ac, 1)
                    a += 1
    return nc


def _run(nc, in_maps, trace=False):
    from concourse.bass_utils import run_bass_kernel_spmd

    return run_bass_kernel_spmd(nc, in_maps, core_ids=list(range(N_CORES)),
                                trace=trace)


def _prep_inputs(x, sa):
    x = np.asarray(x, np.float32).reshape(-1)
    n = x.shape[0]
    xp = np.zeros(NPAD, np.float32)
    xp[:n] = x
    in_maps = []
    for c in range(N_CORES):
        chunk = xp[c * P * FD:(c + 1) * P * FD].reshape(P, FD)
        in_maps.append({"x": chunk})
    return n, in_maps


def kernel(x, knots, F, W, b, mean, _trace=False, _results_out=None):
    consts = _spline_consts(knots, F, W, b, mean)
    n, in_maps = _prep_inputs(x, consts[1])
    nc = _build_nc(consts)
    res = _run(nc, in_maps, trace=_trace)
    if _results_out is not None:
        _results_out.append(res)
    full = np.concatenate([r["out"].reshape(-1) for r in res.results])
    return full[:n].reshape(n, 1).astype(np.float32)
